# revision 15
# baseline (speedup 1.0000x reference)
"""Bidirectional RoPE self-attention (Q is both query and key) on 8 trn2 cores.

Math (per (b,h) pair, T=1024, N=256):
    QR = rope(Q); S = QR @ QR.T / 16; out = softmax(S) @ V

Device strategy (v2 — fp8 end to end on the PE):
  - 96 (b,h) pairs sharded 12-per-core (batch/head parallel, no comm).
  - Host computes rope(Q) in fp32, scales by 1/4 (folds the 1/sqrt(256)
    softmax scale), casts to fp8e4m3 and pre-transposes to the DoubleRow
    [128, 2, T] channel-interleaved layout (elementwise O(TN) input prep,
    like the quantization itself; the DVE read-write-bubble errata makes
    on-device rope ~56us/core, and host rope also halves the Q DMA).
  - scores: one fp8 DoubleRow matmul per (t-tile, s-chunk): K=256 in a
    single pass. Scores land in fp32 PSUM [128, 1024] (2 banks).
  - exp: ScalarE activation per t-tile with a per-head constant bias
    -(max_t |Q_t|^2/16 - 10.4) (host-computed; keeps E <= ~4e4 so it fits
    fp8e5m2 without overflow, and keeps relevant off-diag terms out of
    the subnormal-flush zone). E tiles are written DIRECTLY as fp8e5m2
    into [128, 2, T] chunk tiles whose j slot pairs adjacent s-tiles for
    the DoubleRow moving layout of the second matmul.
  - exp is split across engines: 6 of 8 t-tiles per pair go through the
    ScalarE Exp LUT (fp8e5 RNE out); 2 go through a DVE Schraudolph trick
    (e5m2 bits are linear in log2: uint8(round(5.77*x + B)) viewed as
    fp8e5 IS exp(x) to ~5%) - softmax self-normalization absorbs the
    approximation since numerator and denominator use the same bytes.
  - Z: the softmax denominator must be the sum of the QUANTIZED E values
    actually fed to the matmul (activation accum_out taps pre-quant fp32
    - measured), so the fp8 E tiles are DMA'd out and the host computes
    Z as column sums of exactly the bytes the matmul consumed, then
    applies 1/Z. This keeps normalization exact even with two different
    quantizers in play.
  - attn @ V, transposed: scores/E are symmetric (per-head constant bias
    preserves symmetry), so E tiles [t, s] are also [s, t];
    outT[n, t] = sum_s V8[s, n] E[s, t] with V8 as fp8e4m3 DoubleRow
    stationary and E fp8e5m2 moving: both matmuls run at the fp8 2x rate.
  - V fp8 quantization is repaired on the host: out += dV[t] (dV = V-V8),
    exploiting diag-dominant attention (A_tt ~ 1); residual error
    <= (1-A_tt)*|dV| ~ 1e-2 absolute worst-case. Host also divides by Z.
  - emission is a flat slot schedule with a 2-slot producer->consumer
    skew: slot s runs scores+exp for chunk s and attn@V for chunk s-2,
    so the PE fills its exp-wait time with the next chunk's (or next
    pair's) score matmuls.
"""

from contextlib import ExitStack

import numpy as np

import concourse.bacc as bacc
import concourse.tile as tile
from concourse import mybir

B, NH, T, N = 8, 12, 1024, 256
NCORES = 8
PAIRS = B * NH // NCORES  # 12 (b,h) pairs per core
F32 = mybir.dt.float32
BF16 = mybir.dt.bfloat16
FP8E4 = mybir.dt.float8e4
FP8E5 = mybir.dt.float8e5
EXP = mybir.ActivationFunctionType.Exp
DR = mybir.MatmulPerfMode.DoubleRow

NTT = T // 128   # 8 t-tiles per pair
NCH = NTT // 2   # 4 DoubleRow s-chunks (K=256 each) for attn@V
SHIFT_MARGIN = 10.4
NSCH = 2         # t-tiles per pair quantized via DVE Schraudolph (of NTT)
A8 = float(4.0 / np.log(2.0))
B8 = float(60.0 - 0.5 * 4 * 0.0861 / np.log(2.0))
U8 = mybir.dt.uint8


def build_nc(pairs=PAIRS):
    nc = bacc.Bacc("TRN2", target_bir_lowering=False, debug=False,
                   enable_asserts=False)

    qr = nc.dram_tensor("qr", [pairs, 128, 2, T], FP8E4, kind="ExternalInput")
    v = nc.dram_tensor("v", [pairs, 128, NCH, 2, 2, 128], FP8E4,
                       kind="ExternalInput")
    db = nc.dram_tensor("db", [128, 2 * pairs], F32, kind="ExternalInput")
    outt = nc.dram_tensor("outt", [pairs, 128, 2, T], BF16,
                          kind="ExternalOutput")
    edump = nc.dram_tensor("edump", [pairs, 128, NCH * 2 * T], FP8E5,
                           kind="ExternalOutput")

    with tile.TileContext(nc) as tc, ExitStack() as ctx:
        cpool = ctx.enter_context(tc.tile_pool(name="cs", bufs=1))
        qrpool = ctx.enter_context(tc.tile_pool(name="qr", bufs=3))
        epool = ctx.enter_context(tc.tile_pool(name="e", bufs=2))
        vpool = ctx.enter_context(tc.tile_pool(name="v", bufs=3))
        opool = ctx.enter_context(tc.tile_pool(name="o", bufs=2))
        ps_s = ctx.enter_context(tc.tile_pool(name="ps_s", bufs=2, space="PSUM"))
        ps_o = ctx.enter_context(tc.tile_pool(name="ps_o", bufs=1, space="PSUM"))

        dbt = cpool.tile([128, 2 * pairs], F32, tag="db")
        nc.scalar.dma_start(dbt[:], db[:])

        # warm the PE clock gate (HAM) with dummy matmuls while the first
        # qr/v DMAs are in flight; garbage values, result discarded
        warm = cpool.tile([128, 512], BF16, tag="warm")
        nc.gpsimd.memset(warm[:], 0.0)
        wps = ps_s.tile([128, T], F32, name="ps")
        for _ in range(8):
            nc.tensor.matmul(wps[:, 0:512], warm[:, 0:128], warm[:],
                             start=True, stop=True)

        def load_pair(p):
            qr8 = qrpool.tile([128, 2 * T], FP8E4)
            nc.sync.dma_start(qr8[:].rearrange("p (k t) -> p k t", k=2), qr[p])
            v8 = vpool.tile([128, NCH * 2 * N], FP8E4)
            nc.gpsimd.dma_start(
                v8[:].rearrange("p (c j n m) -> p c j n m", c=NCH, j=2, n=2),
                v[p])
            return qr8, v8

        qrs, v8s, ets, pos, o8s = {}, {}, {}, {}, {}
        qrs[0], v8s[0] = load_pair(0)

        # Flat slot schedule with a producer->consumer skew: slot s emits
        # scores+exp for (p, c) = divmod(s, 4) and attn@V for the chunk
        # produced SKEW slots earlier, so the PE fills its exp-wait time
        # with the next chunks' score matmuls.
        SKEW = 3
        for s in range(4 * pairs + SKEW):
            if s < 4 * pairs:
                p, c = divmod(s, 4)
                if c == 0:
                    if p + 1 < pairs:
                        qrs[p + 1], v8s[p + 1] = load_pair(p + 1)
                    ets[p] = epool.tile([128, NCH * 2 * T], FP8E5,
                                        name="ep", tag="ep")
                qr3 = qrs[p][:].rearrange("p (j t) -> p j t", j=2)
                e2 = ets[p][:, c * 2 * T:(c + 1) * 2 * T]
                for j in range(2):
                    tt = 2 * c + j
                    ps = ps_s.tile([128, T], F32)
                    for sc in range(T // 512):
                        nc.tensor.matmul(
                            ps[:, sc * 512:(sc + 1) * 512],
                            qr3[:, :, tt * 128:(tt + 1) * 128],
                            qr3[:, :, sc * 512:(sc + 1) * 512],
                            start=True, stop=True, perf_mode=DR,
                        )
                    if tt < NSCH:
                        # Schraudolph exp on DVE: e5m2 bits are linear in
                        # log2(E); fp32->uint8 conversion rounds+saturates
                        nc.vector.tensor_scalar(
                            e2[:, j * T:(j + 1) * T].bitcast(U8), ps[:],
                            A8, dbt[:, pairs + p:pairs + p + 1],
                            mybir.AluOpType.mult, mybir.AluOpType.add)
                    else:
                        nc.scalar.activation(e2[:, j * T:(j + 1) * T],
                                             ps[:], EXP,
                                             bias=dbt[:, p:p + 1])
                if c == NCH - 1:
                    # E bytes to the host for the exact-Z column sums
                    eng = nc.gpsimd if p % 2 else nc.scalar
                    eng.dma_start(edump[p], ets[p][:])

            if s >= SKEW:
                p2, c2 = divmod(s - SKEW, 4)
                v5 = v8s[p2][:].rearrange("p (c j n m) -> p c j n m",
                                          c=NCH, j=2, n=2)
                e3 = ets[p2][:, c2 * 2 * T:(c2 + 1) * 2 * T].rearrange(
                    "p (j t) -> p j t", j=2)
                if c2 == 0:
                    pos[p2] = {}
                for nch in range(2):
                    for tch in range(2):
                        key = (nch, tch)
                        if c2 == 0:
                            pos[p2][key] = ps_o.tile([128, 512], F32,
                                                     name=f"po{nch}{tch}",
                                                     tag=f"po{nch}{tch}")
                        nc.tensor.matmul(
                            pos[p2][key][:],
                            v5[:, c2, :, nch, :],
                            e3[:, :, tch * 512:(tch + 1) * 512],
                            start=(c2 == 0), stop=(c2 == NCH - 1),
                            perf_mode=DR,
                        )
                        if c2 == NCH - 1:
                            # drain each po as soon as its accumulation
                            # stops so the banks free up early
                            if (nch, tch) == (0, 0):
                                o8s[p2] = opool.tile([128, 2 * T], BF16,
                                                     name="o8", tag="o8")
                            nc.vector.tensor_copy(
                                o8s[p2][:, nch * T + tch * 512:
                                        nch * T + (tch + 1) * 512],
                                pos[p2][key][:])
                if c2 == NCH - 1:
                    eng = nc.sync if p2 % 2 else nc.scalar
                    eng.dma_start(outt[p2], o8s[p2][:].rearrange(
                        "p (k t) -> p k t", k=2))
                    qrs.pop(p2), v8s.pop(p2), ets.pop(p2), pos.pop(p2)
                    o8s.pop(p2)

    nc.compile()
    return nc


def host_prep(Q, V, freqs):
    """Returns per-core in_maps for the 8 cores + host-side dV."""
    import ml_dtypes
    e4 = ml_dtypes.float8_e4m3

    Q = np.ascontiguousarray(np.asarray(Q), dtype=np.float32)
    V = np.ascontiguousarray(np.asarray(V), dtype=np.float32)
    freqs = np.asarray(freqs, dtype=np.float32)

    # rope on host (fp32), scaled by 1/4 so S lands in PSUM as S/16.
    half = freqs.reshape(-1)[0::2]  # [128] cycles-per-step
    t_col = np.arange(T, dtype=np.float32).reshape(T, 1)
    phases = t_col * half.reshape(1, 128)  # [T, 128] fp32
    ang = np.mod(phases, np.float32(1.0)) * np.float32(2.0 * np.pi)
    C = np.cos(ang).astype(np.float32) * np.float32(0.25)  # [T, 128]
    S = np.sin(ang).astype(np.float32) * np.float32(0.25)

    G = B * NH
    Qg = Q.reshape(G, T, N)
    q0 = Qg[:, :, 0::2]  # even channels [G, T, 128]
    q1 = Qg[:, :, 1::2]
    # QR in DoubleRow [128, 2, T] layout: slot 0 = even-channel rows,
    # slot 1 = odd-channel rows, both transposed to [n, t].
    QR = np.empty((G, 128, 2, T), e4)
    QR[:, :, 0] = (q0 * C - q1 * S).transpose(0, 2, 1).astype(e4)
    QR[:, :, 1] = (q1 * C + q0 * S).transpose(0, 2, 1).astype(e4)

    # per-head exp shift: max_t |Q_t|^2/16 - margin (rope preserves norms)
    dstar = np.einsum('gtn,gtn->gt', Qg, Qg, dtype=np.float64) / 16.0
    shift = (dstar.max(axis=1) - SHIFT_MARGIN).astype(np.float32)  # [G]

    # V8 fp8e4m3 in DoubleRow stationary layout [g, p, c, j, nch, m]
    # (s = 256c + 128j + p, n = 128nch + m); dV = V - V8 stays on host.
    Vg = V.reshape(G, T, N)
    V8 = Vg.astype(e4)
    dV = Vg - V8.astype(np.float32)
    V8l = np.ascontiguousarray(
        V8.reshape(G, NCH, 2, 128, 2, 128).transpose(0, 3, 1, 2, 4, 5))

    in_maps = []
    for cidx in range(NCORES):
        sl = slice(cidx * PAIRS, (cidx + 1) * PAIRS)
        dbc = np.empty((128, 2 * PAIRS), np.float32)
        dbc[:, :PAIRS] = -shift[sl]                      # ScalarE Exp bias
        dbc[:, PAIRS:] = B8 - A8 * shift[sl]             # Schraudolph offset
        in_maps.append({"qr": QR[sl], "v": V8l[sl], "db": dbc})
    return in_maps, dV


_CACHED_NC = None


def kernel(Q, V, freqs):
    global _CACHED_NC
    from concourse.bass_utils import run_bass_kernel_spmd

    in_maps, dV = host_prep(Q, V, freqs)
    if _CACHED_NC is None:
        _CACHED_NC = build_nc()
    res = run_bass_kernel_spmd(_CACHED_NC, in_maps, list(range(NCORES)))
    # outt [pairs, 128 (n%128), 2 (n//128), T] -> [g, T, N]
    full = np.concatenate([res.results[c]["outt"].astype(np.float32)
                           for c in range(NCORES)])
    full = full.transpose(0, 3, 2, 1).reshape(B * NH, T, N)
    # Z[g, t] = sum over stored rows s of the exact fp8 bytes the matmul
    # used: edump [pairs, c, p, (j t)] with s = 256c + 128j + p
    ec = np.concatenate([res.results[c]["edump"] for c in range(NCORES)])
    ef = ec.reshape(B * NH, 128, NCH, 2, T).astype(np.float32)
    Z = ef.sum(axis=(1, 2, 3))
    out = full / Z[:, :, None] + dV
    return np.ascontiguousarray(out.astype(np.float32)).reshape(B, NH, T, N)


# revision 16
# speedup vs baseline: 1.0459x; 1.0459x over previous
"""Bidirectional RoPE self-attention (Q is both query and key) on 8 trn2 cores.

Math (per (b,h) pair, T=1024, N=256):
    QR = rope(Q); S = QR @ QR.T / 16; out = softmax(S) @ V

Device strategy (v2 — fp8 end to end on the PE):
  - 96 (b,h) pairs sharded 12-per-core (batch/head parallel, no comm).
  - Host computes rope(Q) in fp32, scales by 1/4 (folds the 1/sqrt(256)
    softmax scale), casts to fp8e4m3 and pre-transposes to the DoubleRow
    [128, 2, T] channel-interleaved layout (elementwise O(TN) input prep,
    like the quantization itself; the DVE read-write-bubble errata makes
    on-device rope ~56us/core, and host rope also halves the Q DMA).
  - scores: one fp8 DoubleRow matmul per (t-tile, s-chunk): K=256 in a
    single pass. Scores land in fp32 PSUM [128, 1024] (2 banks).
  - exp: ScalarE activation per t-tile with a per-head constant bias
    -(max_t |Q_t|^2/16 - 10.4) (host-computed; keeps E <= ~4e4 so it fits
    fp8e5m2 without overflow, and keeps relevant off-diag terms out of
    the subnormal-flush zone). E tiles are written DIRECTLY as fp8e5m2
    into [128, 2, T] chunk tiles whose j slot pairs adjacent s-tiles for
    the DoubleRow moving layout of the second matmul.
  - exp is split across engines: 6 of 8 t-tiles per pair go through the
    ScalarE Exp LUT (fp8e5 RNE out); 2 go through a DVE Schraudolph trick
    (e5m2 bits are linear in log2: uint8(round(5.77*x + B)) viewed as
    fp8e5 IS exp(x) to ~5%) - softmax self-normalization absorbs the
    approximation since numerator and denominator use the same bytes.
  - Z: the softmax denominator must be the sum of the QUANTIZED E values
    actually fed to the matmul (activation accum_out taps pre-quant fp32
    - measured), so the fp8 E tiles are DMA'd out and the host computes
    Z as column sums of exactly the bytes the matmul consumed, then
    applies 1/Z. This keeps normalization exact even with two different
    quantizers in play.
  - attn @ V, transposed: scores/E are symmetric (per-head constant bias
    preserves symmetry), so E tiles [t, s] are also [s, t];
    outT[n, t] = sum_s V8[s, n] E[s, t] with V8 as fp8e4m3 DoubleRow
    stationary and E fp8e5m2 moving: both matmuls run at the fp8 2x rate.
  - V fp8 quantization is repaired on the host: out += dV[t] (dV = V-V8),
    exploiting diag-dominant attention (A_tt ~ 1); residual error
    <= (1-A_tt)*|dV| ~ 1e-2 absolute worst-case. Host also divides by Z.
  - emission is a flat slot schedule with a 2-slot producer->consumer
    skew: slot s runs scores+exp for chunk s and attn@V for chunk s-2,
    so the PE fills its exp-wait time with the next chunk's (or next
    pair's) score matmuls.
"""

from contextlib import ExitStack

import numpy as np

import concourse.bacc as bacc
import concourse.tile as tile
from concourse import mybir

B, NH, T, N = 8, 12, 1024, 256
NCORES = 8
PAIRS = B * NH // NCORES  # 12 (b,h) pairs per core
F32 = mybir.dt.float32
BF16 = mybir.dt.bfloat16
FP8E4 = mybir.dt.float8e4
FP8E5 = mybir.dt.float8e5
EXP = mybir.ActivationFunctionType.Exp
DR = mybir.MatmulPerfMode.DoubleRow

NTT = T // 128   # 8 t-tiles per pair
NCH = NTT // 2   # 4 DoubleRow s-chunks (K=256 each) for attn@V
SHIFT_MARGIN = 10.4
NSCH = 2         # t-tiles per pair quantized via DVE Schraudolph (of NTT)
A8 = float(4.0 / np.log(2.0))
B8 = float(60.0 - 0.5 * 4 * 0.0861 / np.log(2.0))
U8 = mybir.dt.uint8


def build_nc(pairs=PAIRS):
    nc = bacc.Bacc("TRN2", target_bir_lowering=False, debug=False,
                   enable_asserts=False)

    qr = nc.dram_tensor("qr", [pairs, 128, 2, T], FP8E4, kind="ExternalInput")
    v = nc.dram_tensor("v", [pairs, 128, NCH, 2, 2, 128], FP8E4,
                       kind="ExternalInput")
    db = nc.dram_tensor("db", [128, 2 * pairs], F32, kind="ExternalInput")
    outt = nc.dram_tensor("outt", [pairs, 128, 2, T], BF16,
                          kind="ExternalOutput")
    edump = nc.dram_tensor("edump", [pairs, NCH, 128, 2 * T], FP8E5,
                           kind="ExternalOutput")

    with tile.TileContext(nc) as tc, ExitStack() as ctx:
        cpool = ctx.enter_context(tc.tile_pool(name="cs", bufs=1))
        qrpool = ctx.enter_context(tc.tile_pool(name="qr", bufs=3))
        epool = ctx.enter_context(tc.tile_pool(name="e", bufs=2))
        vpool = ctx.enter_context(tc.tile_pool(name="v", bufs=3))
        opool = ctx.enter_context(tc.tile_pool(name="o", bufs=2))
        ps_s = ctx.enter_context(tc.tile_pool(name="ps_s", bufs=2, space="PSUM"))
        ps_o = ctx.enter_context(tc.tile_pool(name="ps_o", bufs=1, space="PSUM"))

        dbt = cpool.tile([128, 2 * pairs], F32, tag="db")
        nc.scalar.dma_start(dbt[:], db[:])

        # warm the PE clock gate (HAM) with dummy matmuls while the first
        # qr/v DMAs are in flight; garbage values, result discarded
        warm = cpool.tile([128, 512], BF16, tag="warm")
        nc.gpsimd.memset(warm[:], 0.0)
        wps = ps_s.tile([128, T], F32, name="ps")
        for _ in range(8):
            nc.tensor.matmul(wps[:, 0:512], warm[:, 0:128], warm[:],
                             start=True, stop=True)

        def load_pair(p):
            qr8 = qrpool.tile([128, 2 * T], FP8E4)
            nc.sync.dma_start(qr8[:].rearrange("p (k t) -> p k t", k=2), qr[p])
            v8 = vpool.tile([128, NCH * 2 * N], FP8E4)
            nc.gpsimd.dma_start(
                v8[:].rearrange("p (c j n m) -> p c j n m", c=NCH, j=2, n=2),
                v[p])
            return qr8, v8

        qrs, v8s, ets, pos, o8s = {}, {}, {}, {}, {}
        qrs[0], v8s[0] = load_pair(0)

        # Flat slot schedule with a producer->consumer skew: slot s emits
        # scores+exp for (p, c) = divmod(s, 4) and attn@V for the chunk
        # produced SKEW slots earlier, so the PE fills its exp-wait time
        # with the next chunks' score matmuls.
        SKEW = 3
        for s in range(4 * pairs + SKEW):
            if s < 4 * pairs:
                p, c = divmod(s, 4)
                if c == 0:
                    if p + 1 < pairs:
                        qrs[p + 1], v8s[p + 1] = load_pair(p + 1)
                    ets[p] = {}
                qr3 = qrs[p][:].rearrange("p (j t) -> p j t", j=2)
                e2 = epool.tile([128, 2 * T], FP8E5, name=f"e{c}", tag=f"e{c}")
                ets[p][c] = e2
                for j in range(2):
                    tt = 2 * c + j
                    ps = ps_s.tile([128, T], F32)
                    for sc in range(T // 512):
                        nc.tensor.matmul(
                            ps[:, sc * 512:(sc + 1) * 512],
                            qr3[:, :, tt * 128:(tt + 1) * 128],
                            qr3[:, :, sc * 512:(sc + 1) * 512],
                            start=True, stop=True, perf_mode=DR,
                        )
                    if tt < NSCH:
                        # Schraudolph exp on DVE: e5m2 bits are linear in
                        # log2(E); fp32->uint8 conversion rounds+saturates
                        nc.vector.tensor_scalar(
                            e2[:, j * T:(j + 1) * T].bitcast(U8), ps[:],
                            A8, dbt[:, pairs + p:pairs + p + 1],
                            mybir.AluOpType.mult, mybir.AluOpType.add)
                    else:
                        nc.scalar.activation(e2[:, j * T:(j + 1) * T],
                                             ps[:], EXP,
                                             bias=dbt[:, p:p + 1])
                # E bytes to the host for the exact-Z column sums
                eng = (nc.sync, nc.scalar, nc.gpsimd, nc.scalar)[c]
                eng.dma_start(edump[p, c], e2[:])

            if s >= SKEW:
                p2, c2 = divmod(s - SKEW, 4)
                v5 = v8s[p2][:].rearrange("p (c j n m) -> p c j n m",
                                          c=NCH, j=2, n=2)
                e3 = ets[p2][c2][:].rearrange("p (j t) -> p j t", j=2)
                if c2 == 0:
                    pos[p2] = {}
                for nch in range(2):
                    for tch in range(2):
                        key = (nch, tch)
                        if c2 == 0:
                            pos[p2][key] = ps_o.tile([128, 512], F32,
                                                     name=f"po{nch}{tch}",
                                                     tag=f"po{nch}{tch}")
                        nc.tensor.matmul(
                            pos[p2][key][:],
                            v5[:, c2, :, nch, :],
                            e3[:, :, tch * 512:(tch + 1) * 512],
                            start=(c2 == 0), stop=(c2 == NCH - 1),
                            perf_mode=DR,
                        )
                        if c2 == NCH - 1:
                            # drain each po as soon as its accumulation
                            # stops so the banks free up early
                            if (nch, tch) == (0, 0):
                                o8s[p2] = opool.tile([128, 2 * T], BF16,
                                                     name="o8", tag="o8")
                            nc.vector.tensor_copy(
                                o8s[p2][:, nch * T + tch * 512:
                                        nch * T + (tch + 1) * 512],
                                pos[p2][key][:])
                if c2 == NCH - 1:
                    eng = nc.sync if p2 % 2 else nc.scalar
                    eng.dma_start(outt[p2], o8s[p2][:].rearrange(
                        "p (k t) -> p k t", k=2))
                    qrs.pop(p2), v8s.pop(p2), ets.pop(p2), pos.pop(p2)
                    o8s.pop(p2)

    nc.compile()
    return nc


def host_prep(Q, V, freqs):
    """Returns per-core in_maps for the 8 cores + host-side dV."""
    import ml_dtypes
    e4 = ml_dtypes.float8_e4m3

    Q = np.ascontiguousarray(np.asarray(Q), dtype=np.float32)
    V = np.ascontiguousarray(np.asarray(V), dtype=np.float32)
    freqs = np.asarray(freqs, dtype=np.float32)

    # rope on host (fp32), scaled by 1/4 so S lands in PSUM as S/16.
    half = freqs.reshape(-1)[0::2]  # [128] cycles-per-step
    t_col = np.arange(T, dtype=np.float32).reshape(T, 1)
    phases = t_col * half.reshape(1, 128)  # [T, 128] fp32
    ang = np.mod(phases, np.float32(1.0)) * np.float32(2.0 * np.pi)
    C = np.cos(ang).astype(np.float32) * np.float32(0.25)  # [T, 128]
    S = np.sin(ang).astype(np.float32) * np.float32(0.25)

    G = B * NH
    Qg = Q.reshape(G, T, N)
    q0 = Qg[:, :, 0::2]  # even channels [G, T, 128]
    q1 = Qg[:, :, 1::2]
    # QR in DoubleRow [128, 2, T] layout: slot 0 = even-channel rows,
    # slot 1 = odd-channel rows, both transposed to [n, t].
    QR = np.empty((G, 128, 2, T), e4)
    QR[:, :, 0] = (q0 * C - q1 * S).transpose(0, 2, 1).astype(e4)
    QR[:, :, 1] = (q1 * C + q0 * S).transpose(0, 2, 1).astype(e4)

    # per-head exp shift: max_t |Q_t|^2/16 - margin (rope preserves norms)
    dstar = np.einsum('gtn,gtn->gt', Qg, Qg, dtype=np.float64) / 16.0
    shift = (dstar.max(axis=1) - SHIFT_MARGIN).astype(np.float32)  # [G]

    # V8 fp8e4m3 in DoubleRow stationary layout [g, p, c, j, nch, m]
    # (s = 256c + 128j + p, n = 128nch + m); dV = V - V8 stays on host.
    Vg = V.reshape(G, T, N)
    V8 = Vg.astype(e4)
    dV = Vg - V8.astype(np.float32)
    V8l = np.ascontiguousarray(
        V8.reshape(G, NCH, 2, 128, 2, 128).transpose(0, 3, 1, 2, 4, 5))

    in_maps = []
    for cidx in range(NCORES):
        sl = slice(cidx * PAIRS, (cidx + 1) * PAIRS)
        dbc = np.empty((128, 2 * PAIRS), np.float32)
        dbc[:, :PAIRS] = -shift[sl]                      # ScalarE Exp bias
        dbc[:, PAIRS:] = B8 - A8 * shift[sl]             # Schraudolph offset
        in_maps.append({"qr": QR[sl], "v": V8l[sl], "db": dbc})
    return in_maps, dV


_CACHED_NC = None


def kernel(Q, V, freqs):
    global _CACHED_NC
    from concourse.bass_utils import run_bass_kernel_spmd

    in_maps, dV = host_prep(Q, V, freqs)
    if _CACHED_NC is None:
        _CACHED_NC = build_nc()
    res = run_bass_kernel_spmd(_CACHED_NC, in_maps, list(range(NCORES)))
    # outt [pairs, 128 (n%128), 2 (n//128), T] -> [g, T, N]
    full = np.concatenate([res.results[c]["outt"].astype(np.float32)
                           for c in range(NCORES)])
    full = full.transpose(0, 3, 2, 1).reshape(B * NH, T, N)
    # Z[g, t] = sum over stored rows s of the exact fp8 bytes the matmul
    # used: edump [pairs, c, p, (j t)] with s = 256c + 128j + p
    ec = np.concatenate([res.results[c]["edump"] for c in range(NCORES)])
    ef = ec.reshape(B * NH, NCH, 128, 2, T).astype(np.float32)
    Z = ef.sum(axis=(1, 2, 3))
    out = full / Z[:, :, None] + dV
    return np.ascontiguousarray(out.astype(np.float32)).reshape(B, NH, T, N)
